# revision 27
# baseline (speedup 1.0000x reference)
"""GatedLinearRecurrence Trainium2 kernel (8-core SPMD, Bass/Tile).

Strategy: shard (batch=2) x (4 sequence chunks of 1024 tokens) across 8 cores.
Each core processes 1152 tokens: a 128-token warm-up window (re-computed
redundantly; the recurrence decay makes carry-in truncation error ~1e-24)
followed by its 1024 "main" tokens.  No collectives needed.

v2: all matmuls in bf16 (weights pre-converted on host), gate matmul in
fp8e4m3 with DoubleRow perf mode (2 k-tiles per matmul; weights scaled x32 on
host, descaled for free via the sigmoid activation's scale).  z never round-
trips to HBM: silu(z) is kept in SBUF.  norm_b is folded into a per-channel
in_proj bias applied during PSUM evacuation (in_proj(LN(x)) = rstd*(in_proj(
x-mu)) + w1 @ norm_b).  Elementwise chain (conv, gating, scan I/O) runs in
bf16 for 2x DVE throughput; the scan keeps fp32 state internally.

Per-core pipeline (channels-on-partitions, tokens-on-free layout):
  LN(x) [t,d] -> bf16 -> PE-transpose -> x-hatT [d,t] -> in_proj (bf16 mm)
  -> causal depthwise conv (4 shifted tensor_scalar ops) -> silu -> mask
  -> fp8 copy -> gate matmul (fp8 DoubleRow) -> sigmoid -> b=-(1-a)*xc
  -> tensor_tensor_scan (h=-h) -> y*silu(z) -> out_proj (bf16) -> residual
  subtract -> out [t,dm].

The sign trick: scan data1 = (a-1)*x_conv = -b gives -h; -h*silu(z) = -yg;
out = x - matmul(-yg) = x + proj(yg).
"""
import sys

for p in ("/opt/trn_rl_repo", "/root/.axon_site/_ro/trn_rl_repo"):
    if p not in sys.path:
        sys.path.insert(0, p)

import numpy as np
import ml_dtypes

import concourse.bass as bass
import concourse.bacc as bacc
import concourse.tile as tile
import concourse.mybir as mybir
from concourse.bass_utils import run_bass_kernel_spmd
from concourse.masks import make_identity

F32 = mybir.dt.float32
BF16 = mybir.dt.bfloat16
F8 = mybir.dt.float8e4
AF = mybir.ActivationFunctionType
OP = mybir.AluOpType

B, L, D = 2, 4096, 1024
DI = 2048            # d_inner
NT = 1152            # tokens per core (128 warm-up + 1024 main)
W = 128              # warm-up tokens
CHUNK = 1024
NTT = NT // 128      # 9 token tiles
KD = D // 128        # 8 k-tiles over d_model
KC = DI // 128       # 16 k-tiles over d_inner
TC = 384             # matmul N chunk (3 per core)
NTC = NT // TC
EPS = 1e-5
SG = 32.0            # fp8 gate weight scale

_cache = {}


def _build():
    nc = bacc.Bacc(None, target_bir_lowering=False)

    x_h = nc.dram_tensor("x", [NT, D], F32, kind="ExternalInput")
    w1x_h = nc.dram_tensor("w1x", [KC, 128, KD * 128], BF16, kind="ExternalInput")
    w1z_h = nc.dram_tensor("w1z", [KC, 128, KD * 128], BF16, kind="ExternalInput")
    gw_h = nc.dram_tensor("gw", [KC, 128, KC * 128], F8, kind="ExternalInput")
    op_h = nc.dram_tensor("opw", [DI, D], BF16, kind="ExternalInput")
    convw_h = nc.dram_tensor("convw", [128, KC * 4], F32, kind="ExternalInput")
    convb_h = nc.dram_tensor("convb", [128, KC], F32, kind="ExternalInput")
    gateb_h = nc.dram_tensor("gateb", [128, KC], F32, kind="ExternalInput")
    gatebn_h = nc.dram_tensor("gatebn", [128, KC], F32, kind="ExternalInput")
    inbx_h = nc.dram_tensor("inbx", [128, KC], F32, kind="ExternalInput")
    inbz_h = nc.dram_tensor("inbz", [128, KC], F32, kind="ExternalInput")
    mask_h = nc.dram_tensor("mask", [1, NT], BF16, kind="ExternalInput")
    out_h = nc.dram_tensor("out", [CHUNK, D], F32, kind="ExternalOutput")

    with tile.TileContext(nc) as tc:
        with tc.tile_pool(name="consts", bufs=1) as consts:

            ident = consts.tile([128, 128], BF16, name="ident")
            make_identity(nc, ident)
            mask_sb = consts.tile([128, W], BF16, name="mask_sb")
            nc.gpsimd.dma_start(
                out=mask_sb,
                in_=bass.AP(tensor=mask_h, offset=0, ap=[[0, 128], [1, W]]),
            )
            convw = consts.tile([128, KC * 4], F32, name="convw")
            nc.gpsimd.dma_start(out=convw, in_=convw_h.ap())
            convb = consts.tile([128, KC], F32, name="convb")
            nc.gpsimd.dma_start(out=convb, in_=convb_h.ap())
            gateb = consts.tile([128, KC], F32, name="gateb")
            nc.gpsimd.dma_start(out=gateb, in_=gateb_h.ap())
            gatebn = consts.tile([128, KC], F32, name="gatebn")
            nc.gpsimd.dma_start(out=gatebn, in_=gatebn_h.ap())
            inbx = consts.tile([128, KC], F32, name="inbx")
            nc.gpsimd.dma_start(out=inbx, in_=inbx_h.ap())
            inbz = consts.tile([128, KC], F32, name="inbz")
            nc.gpsimd.dma_start(out=inbz, in_=inbz_h.ap())
            eps_t = consts.tile([128, 1], F32, name="eps_t")
            nc.vector.memset(eps_t, EPS)

            with tc.tile_pool(name="xcp", bufs=1) as xcp:
                xc = [xcp.tile([128, NT], BF16, name=f"xct{e}") for e in range(KC)]
                xc8 = xcp.tile([128, KC, NT], F8, name="xc8")
                sz = [xcp.tile([128, NT], BF16, name=f"szt{e}") for e in range(KC)]

                # ---- S1-S3: LN, transpose, in_proj (x & z), conv, silu ----
                with tc.tile_pool(name="xT", bufs=1) as xTp, \
                     tc.tile_pool(name="s1roll", bufs=2) as s1r, \
                     tc.tile_pool(name="stat", bufs=4) as stp, \
                     tc.tile_pool(name="w1s", bufs=3) as ws, \
                     tc.tile_pool(name="psmm", bufs=5, space="PSUM") as psmm, \
                     tc.tile_pool(name="pstr", bufs=2, space="PSUM") as pstr:

                    # x-hat-T chunk tiles [c][:, kt, :]: finer deps, so the
                    # first in_proj matmuls start after 3 LN iterations.
                    xT = [xTp.tile([128, KD, TC], BF16, name=f"xTt{c_}")
                          for c_ in range(NTC)]

                    for it in range(NTT):
                        tc3, col = it // 3, (it % 3) * 128
                        xt = s1r.tile([128, D], F32, tag="xt", bufs=3, name="xt")
                        # split halves so bn_stats starts after half a tile;
                        # first chunk's tiles use two queues to cut the head
                        h2q = nc.scalar if it < 3 else nc.sync
                        nc.sync.dma_start(
                            out=xt[:, 0:512],
                            in_=x_h.ap()[it * 128:(it + 1) * 128, 0:512])
                        h2q.dma_start(
                            out=xt[:, 512:1024],
                            in_=x_h.ap()[it * 128:(it + 1) * 128, 512:1024])
                        stats = stp.tile([128, 2, 6], F32, tag="stats", name="stats")
                        nc.vector.bn_stats(out=stats[:, 0, :], in_=xt[:, 0:512])
                        nc.vector.bn_stats(out=stats[:, 1, :], in_=xt[:, 512:1024])
                        mv = stp.tile([128, 2], F32, tag="mv", name="mv")
                        nc.vector.bn_aggr(out=mv, in_=stats)
                        rstd = stp.tile([128, 1], F32, tag="rstd", name="rstd")
                        nc.scalar.activation(out=rstd, in_=mv[:, 1:2], func=AF.Sqrt,
                                             bias=eps_t, scale=1.0)
                        nc.vector.reciprocal(out=rstd, in_=rstd)
                        xhat = s1r.tile([128, D], BF16, tag="xhat", bufs=3, name="xhat")
                        nc.vector.tensor_scalar(out=xhat, in0=xt, scalar1=mv[:, 0:1],
                                                scalar2=rstd, op0=OP.subtract, op1=OP.mult)
                        for dp in range(KD // 2):
                            pst = pstr.tile([128, 2, 128], BF16, tag="tr", name="pst")
                            nc.tensor.transpose(
                                pst[:, 0, :], xhat[:, dp * 256:dp * 256 + 128], ident)
                            nc.tensor.transpose(
                                pst[:, 1, :], xhat[:, dp * 256 + 128:dp * 256 + 256], ident)
                            nc.scalar.copy(
                                out=xT[tc3][:, dp * 2:dp * 2 + 2, col:col + 128],
                                in_=pst)

                    # in_proj x-half + conv + silu + warm-up mask + fp8 copy.
                    # The first 4 ets interleave their t-chunks so the PE
                    # never waits on layernorm chunks still in flight.
                    NW = 4
                    order = [(e, c) for c in range(NTC) for e in range(NW)]
                    order += [(e, c) for e in range(NW, KC) for c in range(NTC)]
                    wts, xins = {}, {}

                    def s2_chain(et, tc3):
                        if tc3 == 0:
                            wt = ws.tile([128, KD, 128], BF16, tag="w1",
                                         bufs=6, name=f"wt{et}")
                            nc.gpsimd.dma_start(out=wt, in_=w1x_h.ap()[et])
                            wts[et] = wt
                            xin = s1r.tile([128, NT + 3], BF16, tag="xin",
                                           bufs=NW + 2, name=f"xin{et}")
                            nc.vector.memset(xin[:, 0:3], 0.0)
                            xins[et] = xin
                        ps = psmm.tile([128, TC], F32, tag="mm", name="ps")
                        for kt in range(KD):
                            nc.tensor.matmul(
                                ps, wts[et][:, kt, :], xT[tc3][:, kt, :],
                                start=(kt == 0), stop=(kt == KD - 1))
                        nc.scalar.activation(
                            out=xins[et][:, 3 + tc3 * TC: 3 + (tc3 + 1) * TC],
                            in_=ps, func=AF.Identity,
                            bias=inbx[:, et:et + 1], scale=1.0)
                        if tc3 == NTC - 1:
                            xin = xins.pop(et)
                            tmp = s1r.tile([128, NT], BF16, tag="ctmp", name="ctmp")
                            nc.vector.tensor_scalar_mul(
                                tmp, xin[:, 0:NT], convw[:, et * 4:et * 4 + 1])
                            for k in range(1, 4):
                                nc.vector.scalar_tensor_tensor(
                                    out=tmp, in0=xin[:, k:k + NT],
                                    scalar=convw[:, et * 4 + k:et * 4 + k + 1],
                                    in1=tmp, op0=OP.mult, op1=OP.add)
                            # split silu so the warm-up mask multiply is not
                            # an aliased in-place op (those run ~5x slower)
                            nc.scalar.activation(
                                out=xc[et][:, W:], in_=tmp[:, W:], func=AF.Silu,
                                bias=convb[:, et:et + 1], scale=1.0)
                            tsw = stp.tile([128, W], BF16, tag="tsw", name="tsw")
                            nc.scalar.activation(
                                out=tsw, in_=tmp[:, 0:W], func=AF.Silu,
                                bias=convb[:, et:et + 1], scale=1.0)
                            nc.vector.tensor_mul(xc[et][:, 0:W], tsw, mask_sb)
                            nc.scalar.copy(out=xc8[:, et, :], in_=xc[et])

                    for et, tc3 in order:
                        s2_chain(et, tc3)

                    # in_proj z-half; silu rides the PSUM evacuation -> SBUF
                    for et in range(KC):
                        wt = ws.tile([128, KD, 128], BF16, tag="w1", bufs=6, name="wtz")
                        nc.gpsimd.dma_start(out=wt, in_=w1z_h.ap()[et])
                        for tc3 in range(NTC):
                            ps = psmm.tile([128, TC], F32, tag="mm", name="psz")
                            for kt in range(KD):
                                nc.tensor.matmul(
                                    ps, wt[:, kt, :], xT[tc3][:, kt, :],
                                    start=(kt == 0), stop=(kt == KD - 1))
                            nc.scalar.activation(
                                out=sz[et][:, tc3 * TC:(tc3 + 1) * TC], in_=ps,
                                func=AF.Silu, bias=inbz[:, et:et + 1], scale=1.0)

                # ---- S4-S6: gate matmul (fp8 DoubleRow, weight-stationary
                # over the 3 t-chunks), sigmoid (descale x32 rides on it),
                # chunked scan into persistent yg tiles, y*silu(z) in place.
                with tc.tile_pool(name="yp", bufs=1) as yp:
                    yg = [yp.tile([128, NT], BF16, name=f"yg{e}") for e in range(KC)]
                    with tc.tile_pool(name="gws", bufs=5) as gs, \
                         tc.tile_pool(name="ach", bufs=12) as ayp, \
                         tc.tile_pool(name="s6roll", bufs=6) as s6r, \
                         tc.tile_pool(name="psg", bufs=8, space="PSUM") as psg:

                        for et in range(KC):
                            gt = gs.tile([128, KC, 128], F8, tag="gw", name="gt")
                            nc.sync.dma_start(out=gt, in_=gw_h.ap()[et])
                            pss = [psg.tile([128, TC], F32, tag="mm", name="psgt")
                                   for _ in range(NTC)]
                            for kp in range(KC // 2):
                                for tc3 in range(NTC):
                                    nc.tensor.matmul(
                                        pss[tc3], gt[:, 2 * kp:2 * kp + 2, :],
                                        xc8[:, 2 * kp:2 * kp + 2,
                                            tc3 * TC:(tc3 + 1) * TC],
                                        start=(kp == 0), stop=(kp == KC // 2 - 1),
                                        perf_mode=mybir.MatmulPerfMode.DoubleRow)
                            scan_eng = nc.vector
                            ys = s6r.tile([128, NT], BF16, tag="ys", bufs=4,
                                          name="ys")
                            for tc3 in range(NTC):
                                a_t = ayp.tile([128, TC], BF16, tag="ach", name="ach")
                                nc.scalar.activation(
                                    out=a_t, in_=pss[tc3], func=AF.Sigmoid,
                                    bias=gateb[:, et:et + 1], scale=1.0 / SG)
                                # 1-a = sigmoid(-(g)): second ACT evac avoids a
                                # (1-a) subtract on DVE
                                am1 = ayp.tile([128, TC], BF16, tag="am1", name="am1")
                                nc.scalar.activation(
                                    out=am1, in_=pss[tc3], func=AF.Sigmoid,
                                    bias=gatebn[:, et:et + 1], scale=-1.0 / SG)
                                bt = s6r.tile([128, TC], BF16, tag="bt", name="bt")
                                nc.vector.tensor_mul(
                                    bt, am1, xc[et][:, tc3 * TC:(tc3 + 1) * TC])
                                init = (0.0 if tc3 == 0
                                        else ys[:, tc3 * TC - 1:tc3 * TC])
                                scan_eng.tensor_tensor_scan(
                                    out=ys[:, tc3 * TC:(tc3 + 1) * TC],
                                    data0=a_t, data1=bt, initial=init,
                                    op0=OP.mult, op1=OP.add)
                            # yg = y * silu(z), non-aliased for DVE fast mode
                            nc.vector.tensor_mul(
                                yg[et][:, W:], ys[:, W:], sz[et][:, W:])

                    # ---- S7: out_proj + residual.  yg column slices are the
                    # stationary operands; kt-major accumulation, two d-half
                    # passes of 8 PSUM banks; opt streamed per (pass, kt). ----
                    NTB = CHUNK // 128

                    dmaq = [nc.sync, nc.scalar, nc.gpsimd]
                    with tc.tile_pool(name="ops", bufs=8) as opp, \
                         tc.tile_pool(name="s7roll", bufs=6) as s7r, \
                         tc.tile_pool(name="s7res", bufs=16) as s7x, \
                         tc.tile_pool(name="psop", bufs=8, space="PSUM") as psop:
                        xres = {}
                        for nb in range(2):
                            for tb in range(NTB):
                                xres[nb, tb] = s7x.tile([128, 512], F32, tag="xres",
                                                        name=f"xres{nb}_{tb}")
                                # gpsimd queue is idle here; keep sync free
                                # for the opt weight stream
                                nc.gpsimd.dma_start(
                                    out=xres[nb, tb],
                                    in_=x_h.ap()[W + tb * 128:W + (tb + 1) * 128,
                                                 nb * 512:(nb + 1) * 512])
                        # tb-outer: each token tile's accumulation completes
                        # early so residual-add + store pipeline with the
                        # remaining matmuls (no serial tail).  opt tiles for
                        # the current d-half stay resident (16 bufs).
                        for nb in range(2):
                            opts = []
                            for kt in range(KC):
                                opt = opp.tile([128, 512], BF16, tag="opw",
                                               bufs=16, name=f"opt{kt}")
                                nc.sync.dma_start(
                                    out=opt,
                                    in_=op_h.ap()[kt * 128:(kt + 1) * 128,
                                                  nb * 512:(nb + 1) * 512])
                                opts.append(opt)
                            for tb in range(NTB):
                                ps = psop.tile([128, 512], F32, tag="op", name="pso")
                                col = W + tb * 128
                                for kt in range(KC):
                                    nc.tensor.matmul(
                                        ps, yg[kt][:, col:col + 128], opts[kt],
                                        start=(kt == 0), stop=(kt == KC - 1))
                                oh = s7r.tile([128, 512], F32, tag="oh", name="oh")
                                nc.vector.tensor_add(oh, xres[nb, tb], ps)
                                dmaq[1 + tb % 2].dma_start(
                                    out=out_h.ap()[tb * 128:(tb + 1) * 128,
                                                   nb * 512:(nb + 1) * 512],
                                    in_=oh)

    nc.compile()
    return nc


def _prep_host(x, norm_w, norm_b, in_proj_w, conv_w, conv_b, gate_w, gate_b,
               out_proj_w):
    w1 = (in_proj_w * norm_w[None, :]).astype(np.float32)
    inb = (w1 @ norm_b.astype(np.float32)).astype(np.float32)   # [2*DI]

    def rearr(wT, dt, scale=1.0):
        # wT: [K, DI] -> per et slice [K, 128] -> [128, K//128, 128]
        k = wT.shape[0]
        out = np.empty((KC, 128, (k // 128) * 128), dt)
        for et in range(KC):
            s = (wT[:, et * 128:(et + 1) * 128] * scale).astype(dt)
            out[et] = s.reshape(k // 128, 128, 128).transpose(1, 0, 2).reshape(128, -1)
        return np.ascontiguousarray(out)

    w1xT = np.ascontiguousarray(w1[:DI].T)           # [D, DI]
    w1zT = np.ascontiguousarray(w1[DI:].T)           # [D, DI]
    w1x_r = rearr(w1xT, ml_dtypes.bfloat16)
    w1z_r = rearr(w1zT, ml_dtypes.bfloat16)
    gw_r = rearr(np.ascontiguousarray(gate_w.T), ml_dtypes.float8_e4m3, SG)
    op_r = np.ascontiguousarray(out_proj_w.T.astype(ml_dtypes.bfloat16))  # [DI, D]
    convw_r = np.ascontiguousarray(
        conv_w.reshape(KC, 128, 4).transpose(1, 0, 2).reshape(128, KC * 4))
    convb_r = np.ascontiguousarray(conv_b.reshape(KC, 128).T)
    gateb_r = np.ascontiguousarray(gate_b.reshape(KC, 128).T)
    gatebn_r = np.ascontiguousarray(-gateb_r)
    inbx_r = np.ascontiguousarray(inb[:DI].reshape(KC, 128).T)
    inbz_r = np.ascontiguousarray(inb[DI:].reshape(KC, 128).T)

    in_maps = []
    for core in range(8):
        b, j = core // 4, core % 4
        xs = np.zeros((NT, D), np.float32)
        start = j * CHUNK - W
        mask = np.ones((1, NT), ml_dtypes.bfloat16)
        if j == 0:
            xs[W:] = x[b, 0:CHUNK]
            mask[0, :W] = 0.0
        else:
            xs[:] = x[b, start:start + NT]
        in_maps.append({
            "x": np.ascontiguousarray(xs), "w1x": w1x_r, "w1z": w1z_r,
            "gw": gw_r, "opw": op_r, "convw": convw_r, "convb": convb_r,
            "gateb": gateb_r, "gatebn": gatebn_r,
            "inbx": inbx_r, "inbz": inbz_r, "mask": mask,
        })
    return in_maps


def kernel(x, norm_w, norm_b, in_proj_w, conv_w, conv_b, gate_w, gate_b,
           out_proj_w, _trace=False, _collect=None):
    x = np.asarray(x, np.float32)
    if "nc" not in _cache:
        _cache["nc"] = _build()
    nc = _cache["nc"]
    in_maps = _prep_host(
        x, np.asarray(norm_w, np.float32), np.asarray(norm_b, np.float32),
        np.asarray(in_proj_w, np.float32), np.asarray(conv_w, np.float32),
        np.asarray(conv_b, np.float32), np.asarray(gate_w, np.float32),
        np.asarray(gate_b, np.float32), np.asarray(out_proj_w, np.float32))
    res = run_bass_kernel_spmd(nc, in_maps, core_ids=list(range(8)), trace=_trace)
    if _collect is not None:
        _collect.append(res)
    out = np.empty((B, L, D), np.float32)
    for core in range(8):
        b, j = core // 4, core % 4
        out[b, j * CHUNK:(j + 1) * CHUNK] = res.results[core]["out"]
    return out


# revision 31
# speedup vs baseline: 1.0880x; 1.0880x over previous
"""GatedLinearRecurrence Trainium2 kernel (8-core SPMD, Bass/Tile).

Strategy: shard (batch=2) x (4 sequence chunks of 1024 tokens) across 8 cores.
Each core processes 1152 tokens: a 128-token warm-up window (re-computed
redundantly; the recurrence decay makes carry-in truncation error ~1e-24)
followed by its 1024 "main" tokens.  No collectives needed.

v2: all matmuls in bf16 (weights pre-converted on host), gate matmul in
fp8e4m3 with DoubleRow perf mode (2 k-tiles per matmul; weights scaled x32 on
host, descaled for free via the sigmoid activation's scale).  z never round-
trips to HBM: silu(z) is kept in SBUF.  norm_b is folded into a per-channel
in_proj bias applied during PSUM evacuation (in_proj(LN(x)) = rstd*(in_proj(
x-mu)) + w1 @ norm_b).  Elementwise chain (conv, gating, scan I/O) runs in
bf16 for 2x DVE throughput; the scan keeps fp32 state internally.

Per-core pipeline (channels-on-partitions, tokens-on-free layout):
  LN(x) [t,d] -> bf16 -> PE-transpose -> x-hatT [d,t] -> in_proj (bf16 mm)
  -> causal depthwise conv (4 shifted tensor_scalar ops) -> silu -> mask
  -> fp8 copy -> gate matmul (fp8 DoubleRow) -> sigmoid -> b=-(1-a)*xc
  -> tensor_tensor_scan (h=-h) -> y*silu(z) -> out_proj (bf16) -> residual
  subtract -> out [t,dm].

The sign trick: scan data1 = (a-1)*x_conv = -b gives -h; -h*silu(z) = -yg;
out = x - matmul(-yg) = x + proj(yg).
"""
import sys

for p in ("/opt/trn_rl_repo", "/root/.axon_site/_ro/trn_rl_repo"):
    if p not in sys.path:
        sys.path.insert(0, p)

import numpy as np
import ml_dtypes

import concourse.bass as bass
import concourse.bacc as bacc
import concourse.tile as tile
import concourse.mybir as mybir
from concourse.bass_utils import run_bass_kernel_spmd
from concourse.masks import make_identity

F32 = mybir.dt.float32
BF16 = mybir.dt.bfloat16
F8 = mybir.dt.float8e4
AF = mybir.ActivationFunctionType
OP = mybir.AluOpType

B, L, D = 2, 4096, 1024
DI = 2048            # d_inner
NT = 1152            # tokens per core (128 warm-up + 1024 main)
W = 128              # warm-up tokens
CHUNK = 1024
NTT = NT // 128      # 9 token tiles
KD = D // 128        # 8 k-tiles over d_model
KC = DI // 128       # 16 k-tiles over d_inner
TC = 384             # matmul N chunk (3 per core)
NTC = NT // TC
EPS = 1e-5
SG = 32.0            # fp8 gate weight scale

_cache = {}


def _build():
    nc = bacc.Bacc(None, target_bir_lowering=False)

    x_h = nc.dram_tensor("x", [NT, D], F32, kind="ExternalInput")
    w1x_h = nc.dram_tensor("w1x", [KC, 128, KD * 128], BF16, kind="ExternalInput")
    w1z_h = nc.dram_tensor("w1z", [KC, 128, KD * 128], BF16, kind="ExternalInput")
    gw_h = nc.dram_tensor("gw", [KC, 128, KC * 128], F8, kind="ExternalInput")
    op_h = nc.dram_tensor("opw", [DI, D], BF16, kind="ExternalInput")
    convw_h = nc.dram_tensor("convw", [128, KC * 4], F32, kind="ExternalInput")
    convb_h = nc.dram_tensor("convb", [128, KC], F32, kind="ExternalInput")
    gateb_h = nc.dram_tensor("gateb", [128, KC], F32, kind="ExternalInput")
    gatebn_h = nc.dram_tensor("gatebn", [128, KC], F32, kind="ExternalInput")
    inbx_h = nc.dram_tensor("inbx", [128, KC], F32, kind="ExternalInput")
    inbz_h = nc.dram_tensor("inbz", [128, KC], F32, kind="ExternalInput")
    mask_h = nc.dram_tensor("mask", [1, NT], BF16, kind="ExternalInput")
    out_h = nc.dram_tensor("out", [CHUNK, D], F32, kind="ExternalOutput")

    with tile.TileContext(nc) as tc:
        with tc.tile_pool(name="consts", bufs=1) as consts:

            ident = consts.tile([128, 128], BF16, name="ident")
            make_identity(nc, ident)
            mask_sb = consts.tile([128, W], BF16, name="mask_sb")
            nc.gpsimd.dma_start(
                out=mask_sb,
                in_=bass.AP(tensor=mask_h, offset=0, ap=[[0, 128], [1, W]]),
            )
            convw = consts.tile([128, KC * 4], F32, name="convw")
            nc.gpsimd.dma_start(out=convw, in_=convw_h.ap())
            convb = consts.tile([128, KC], F32, name="convb")
            nc.gpsimd.dma_start(out=convb, in_=convb_h.ap())
            gateb = consts.tile([128, KC], F32, name="gateb")
            nc.gpsimd.dma_start(out=gateb, in_=gateb_h.ap())
            gatebn = consts.tile([128, KC], F32, name="gatebn")
            nc.gpsimd.dma_start(out=gatebn, in_=gatebn_h.ap())
            inbx = consts.tile([128, KC], F32, name="inbx")
            nc.gpsimd.dma_start(out=inbx, in_=inbx_h.ap())
            inbz = consts.tile([128, KC], F32, name="inbz")
            nc.gpsimd.dma_start(out=inbz, in_=inbz_h.ap())
            eps_t = consts.tile([128, 1], F32, name="eps_t")
            nc.vector.memset(eps_t, EPS)

            with tc.tile_pool(name="xcp", bufs=1) as xcp:
                xc = [xcp.tile([128, NT], BF16, name=f"xct{e}") for e in range(KC)]
                xc8 = xcp.tile([128, KC, NT], F8, name="xc8")
                sz = [xcp.tile([128, NT], BF16, name=f"szt{e}") for e in range(KC)]

                # ---- S1-S3: LN, transpose, in_proj (x & z), conv, silu ----
                with tc.tile_pool(name="xT", bufs=1) as xTp, \
                     tc.tile_pool(name="s1roll", bufs=2) as s1r, \
                     tc.tile_pool(name="stat", bufs=4) as stp, \
                     tc.tile_pool(name="w1s", bufs=3) as ws, \
                     tc.tile_pool(name="psmm", bufs=5, space="PSUM") as psmm, \
                     tc.tile_pool(name="pstr", bufs=2, space="PSUM") as pstr:

                    # x-hat-T chunk tiles [c][:, kt, :]: finer deps, so the
                    # first in_proj matmuls start after 3 LN iterations.
                    xT = [xTp.tile([128, KD, TC], BF16, name=f"xTt{c_}")
                          for c_ in range(NTC)]

                    for it in range(NTT):
                        tc3, col = it // 3, (it % 3) * 128
                        xt = s1r.tile([128, D], F32, tag="xt", bufs=3, name="xt")
                        # split halves so bn_stats starts after half a tile
                        nc.sync.dma_start(
                            out=xt[:, 0:512],
                            in_=x_h.ap()[it * 128:(it + 1) * 128, 0:512])
                        nc.sync.dma_start(
                            out=xt[:, 512:1024],
                            in_=x_h.ap()[it * 128:(it + 1) * 128, 512:1024])
                        stats = stp.tile([128, 2, 6], F32, tag="stats", name="stats")
                        nc.vector.bn_stats(out=stats[:, 0, :], in_=xt[:, 0:512])
                        nc.vector.bn_stats(out=stats[:, 1, :], in_=xt[:, 512:1024])
                        mv = stp.tile([128, 2], F32, tag="mv", name="mv")
                        nc.vector.bn_aggr(out=mv, in_=stats)
                        rstd = stp.tile([128, 1], F32, tag="rstd", name="rstd")
                        nc.scalar.activation(out=rstd, in_=mv[:, 1:2], func=AF.Sqrt,
                                             bias=eps_t, scale=1.0)
                        nc.vector.reciprocal(out=rstd, in_=rstd)
                        xhat = s1r.tile([128, D], BF16, tag="xhat", bufs=3, name="xhat")
                        nc.vector.tensor_scalar(out=xhat, in0=xt, scalar1=mv[:, 0:1],
                                                scalar2=rstd, op0=OP.subtract, op1=OP.mult)
                        for dp in range(KD // 2):
                            pst = pstr.tile([128, 2, 128], BF16, tag="tr", name="pst")
                            nc.tensor.transpose(
                                pst[:, 0, :], xhat[:, dp * 256:dp * 256 + 128], ident)
                            nc.tensor.transpose(
                                pst[:, 1, :], xhat[:, dp * 256 + 128:dp * 256 + 256], ident)
                            nc.scalar.copy(
                                out=xT[tc3][:, dp * 2:dp * 2 + 2, col:col + 128],
                                in_=pst)

                    # in_proj x-half + conv + silu + warm-up mask + fp8 copy.
                    # The first 4 ets interleave their t-chunks so the PE
                    # never waits on layernorm chunks still in flight.
                    NW = 4
                    order = [(e, c) for c in range(NTC) for e in range(NW)]
                    order += [(e, c) for e in range(NW, KC) for c in range(NTC)]
                    wts, xins = {}, {}

                    def s2_chain(et, tc3):
                        if tc3 == 0:
                            wt = ws.tile([128, KD, 128], BF16, tag="w1",
                                         bufs=6, name=f"wt{et}")
                            nc.gpsimd.dma_start(out=wt, in_=w1x_h.ap()[et])
                            wts[et] = wt
                            xin = s1r.tile([128, NT + 3], BF16, tag="xin",
                                           bufs=NW + 2, name=f"xin{et}")
                            nc.vector.memset(xin[:, 0:3], 0.0)
                            xins[et] = xin
                        ps = psmm.tile([128, TC], F32, tag="mm", name="ps")
                        for kt in range(KD):
                            nc.tensor.matmul(
                                ps, wts[et][:, kt, :], xT[tc3][:, kt, :],
                                start=(kt == 0), stop=(kt == KD - 1))
                        nc.scalar.activation(
                            out=xins[et][:, 3 + tc3 * TC: 3 + (tc3 + 1) * TC],
                            in_=ps, func=AF.Identity,
                            bias=inbx[:, et:et + 1], scale=1.0)
                        if tc3 == NTC - 1:
                            xin = xins.pop(et)
                            tmp = s1r.tile([128, NT], BF16, tag="ctmp", name="ctmp")
                            nc.vector.tensor_scalar_mul(
                                tmp, xin[:, 0:NT], convw[:, et * 4:et * 4 + 1])
                            for k in range(1, 4):
                                nc.vector.scalar_tensor_tensor(
                                    out=tmp, in0=xin[:, k:k + NT],
                                    scalar=convw[:, et * 4 + k:et * 4 + k + 1],
                                    in1=tmp, op0=OP.mult, op1=OP.add)
                            # split silu so the warm-up mask multiply is not
                            # an aliased in-place op (those run ~5x slower)
                            nc.scalar.activation(
                                out=xc[et][:, W:], in_=tmp[:, W:], func=AF.Silu,
                                bias=convb[:, et:et + 1], scale=1.0)
                            tsw = stp.tile([128, W], BF16, tag="tsw", name="tsw")
                            nc.scalar.activation(
                                out=tsw, in_=tmp[:, 0:W], func=AF.Silu,
                                bias=convb[:, et:et + 1], scale=1.0)
                            nc.vector.tensor_mul(xc[et][:, 0:W], tsw, mask_sb)
                            nc.scalar.copy(out=xc8[:, et, :], in_=xc[et])

                    for et, tc3 in order:
                        s2_chain(et, tc3)

                    # in_proj z-half; silu rides the PSUM evacuation -> SBUF
                    for et in range(KC):
                        wt = ws.tile([128, KD, 128], BF16, tag="w1", bufs=6, name="wtz")
                        nc.gpsimd.dma_start(out=wt, in_=w1z_h.ap()[et])
                        for tc3 in range(NTC):
                            ps = psmm.tile([128, TC], F32, tag="mm", name="psz")
                            for kt in range(KD):
                                nc.tensor.matmul(
                                    ps, wt[:, kt, :], xT[tc3][:, kt, :],
                                    start=(kt == 0), stop=(kt == KD - 1))
                            nc.scalar.activation(
                                out=sz[et][:, tc3 * TC:(tc3 + 1) * TC], in_=ps,
                                func=AF.Silu, bias=inbz[:, et:et + 1], scale=1.0)

                # ---- S4-S6: gate matmul (fp8 DoubleRow, weight-stationary
                # over the 3 t-chunks), sigmoid (descale x32 rides on it),
                # chunked scan into persistent yg tiles, y*silu(z) in place.
                with tc.tile_pool(name="yp", bufs=1) as yp:
                    yg = [yp.tile([128, NT], BF16, name=f"yg{e}") for e in range(KC)]
                    with tc.tile_pool(name="gws", bufs=5) as gs, \
                         tc.tile_pool(name="ach", bufs=12) as ayp, \
                         tc.tile_pool(name="s6roll", bufs=6) as s6r, \
                         tc.tile_pool(name="psg", bufs=8, space="PSUM") as psg:

                        for et in range(KC):
                            gt = gs.tile([128, KC, 128], F8, tag="gw", name="gt")
                            nc.sync.dma_start(out=gt, in_=gw_h.ap()[et])
                            pss = [psg.tile([128, TC], F32, tag="mm", name="psgt")
                                   for _ in range(NTC)]
                            for kp in range(KC // 2):
                                for tc3 in range(NTC):
                                    nc.tensor.matmul(
                                        pss[tc3], gt[:, 2 * kp:2 * kp + 2, :],
                                        xc8[:, 2 * kp:2 * kp + 2,
                                            tc3 * TC:(tc3 + 1) * TC],
                                        start=(kp == 0), stop=(kp == KC // 2 - 1),
                                        perf_mode=mybir.MatmulPerfMode.DoubleRow)
                            scan_eng = nc.vector
                            ys = s6r.tile([128, NT], BF16, tag="ys", bufs=4,
                                          name="ys")
                            for tc3 in range(NTC):
                                a_t = ayp.tile([128, TC], BF16, tag="ach", name="ach")
                                nc.scalar.activation(
                                    out=a_t, in_=pss[tc3], func=AF.Sigmoid,
                                    bias=gateb[:, et:et + 1], scale=1.0 / SG)
                                # 1-a = sigmoid(-(g)): second ACT evac avoids a
                                # (1-a) subtract on DVE
                                am1 = ayp.tile([128, TC], BF16, tag="am1", name="am1")
                                nc.scalar.activation(
                                    out=am1, in_=pss[tc3], func=AF.Sigmoid,
                                    bias=gatebn[:, et:et + 1], scale=-1.0 / SG)
                                bt = s6r.tile([128, TC], BF16, tag="bt", name="bt")
                                nc.vector.tensor_mul(
                                    bt, am1, xc[et][:, tc3 * TC:(tc3 + 1) * TC])
                                init = (0.0 if tc3 == 0
                                        else ys[:, tc3 * TC - 1:tc3 * TC])
                                scan_eng.tensor_tensor_scan(
                                    out=ys[:, tc3 * TC:(tc3 + 1) * TC],
                                    data0=a_t, data1=bt, initial=init,
                                    op0=OP.mult, op1=OP.add)
                            # yg = y * silu(z), non-aliased for DVE fast mode
                            nc.vector.tensor_mul(
                                yg[et][:, W:], ys[:, W:], sz[et][:, W:])

                    # ---- S7: out_proj + residual.  yg column slices are the
                    # stationary operands; kt-major accumulation, two d-half
                    # passes of 8 PSUM banks; opt streamed per (pass, kt). ----
                    NTB = CHUNK // 128

                    dmaq = [nc.sync, nc.scalar, nc.gpsimd]
                    with tc.tile_pool(name="ops", bufs=8) as opp, \
                         tc.tile_pool(name="s7roll", bufs=6) as s7r, \
                         tc.tile_pool(name="s7res", bufs=8) as s7x, \
                         tc.tile_pool(name="psop", bufs=8, space="PSUM") as psop:
                        xres = {}

                        def load_xres(nb):
                            for tb in range(NTB):
                                xres[nb, tb] = s7x.tile([128, 512], F32, tag="xres",
                                                        name=f"xres{nb}_{tb}")
                                # gpsimd queue is idle here; keep sync free
                                # for the opt weight stream
                                nc.gpsimd.dma_start(
                                    out=xres[nb, tb],
                                    in_=x_h.ap()[W + tb * 128:W + (tb + 1) * 128,
                                                 nb * 512:(nb + 1) * 512])

                        load_xres(0)
                        # nb=0 runs kt-outer (each opt tile needed just-in-
                        # time, so the weight stream pipelines with compute);
                        # nb=1 runs tb-outer (its opts prefetch during nb=0)
                        # so the final adds + stores overlap the matmuls and
                        # there is no serial tail.
                        opts1 = []
                        for kt in range(KC):
                            opt = opp.tile([128, 512], BF16, tag="opw",
                                           bufs=32, name=f"opt0_{kt}")
                            nc.sync.dma_start(
                                out=opt, in_=op_h.ap()[kt * 128:(kt + 1) * 128, 0:512])
                            opts1.append(opt)
                        pss = [psop.tile([128, 512], F32, tag="op",
                                         name=f"pso{tb}") for tb in range(NTB)]
                        for kt in range(KC):
                            for tb in range(NTB):
                                col = W + tb * 128
                                nc.tensor.matmul(
                                    pss[tb], yg[kt][:, col:col + 128], opts1[kt],
                                    start=(kt == 0), stop=(kt == KC - 1))
                        opts2 = []
                        for kt in range(KC):
                            opt = opp.tile([128, 512], BF16, tag="opw",
                                           bufs=32, name=f"opt1_{kt}")
                            nc.sync.dma_start(
                                out=opt,
                                in_=op_h.ap()[kt * 128:(kt + 1) * 128, 512:1024])
                            opts2.append(opt)
                        load_xres(1)
                        for tb in range(NTB):
                            oh = s7r.tile([128, 512], F32, tag="oh", name="oh")
                            nc.vector.tensor_add(oh, xres[0, tb], pss[tb])
                            dmaq[1 + tb % 2].dma_start(
                                out=out_h.ap()[tb * 128:(tb + 1) * 128, 0:512],
                                in_=oh)
                        for tb in range(NTB):
                            ps = psop.tile([128, 512], F32, tag="op", name="pso2")
                            col = W + tb * 128
                            for kt in range(KC):
                                nc.tensor.matmul(
                                    ps, yg[kt][:, col:col + 128], opts2[kt],
                                    start=(kt == 0), stop=(kt == KC - 1))
                            oh = s7r.tile([128, 512], F32, tag="oh", name="oh")
                            nc.vector.tensor_add(oh, xres[1, tb], ps)
                            dmaq[1 + tb % 2].dma_start(
                                out=out_h.ap()[tb * 128:(tb + 1) * 128, 512:1024],
                                in_=oh)

    nc.compile()
    return nc


def _prep_host(x, norm_w, norm_b, in_proj_w, conv_w, conv_b, gate_w, gate_b,
               out_proj_w):
    w1 = (in_proj_w * norm_w[None, :]).astype(np.float32)
    inb = (w1 @ norm_b.astype(np.float32)).astype(np.float32)   # [2*DI]

    def rearr(wT, dt, scale=1.0):
        # wT: [K, DI] -> per et slice [K, 128] -> [128, K//128, 128]
        k = wT.shape[0]
        out = np.empty((KC, 128, (k // 128) * 128), dt)
        for et in range(KC):
            s = (wT[:, et * 128:(et + 1) * 128] * scale).astype(dt)
            out[et] = s.reshape(k // 128, 128, 128).transpose(1, 0, 2).reshape(128, -1)
        return np.ascontiguousarray(out)

    w1xT = np.ascontiguousarray(w1[:DI].T)           # [D, DI]
    w1zT = np.ascontiguousarray(w1[DI:].T)           # [D, DI]
    w1x_r = rearr(w1xT, ml_dtypes.bfloat16)
    w1z_r = rearr(w1zT, ml_dtypes.bfloat16)
    gw_r = rearr(np.ascontiguousarray(gate_w.T), ml_dtypes.float8_e4m3, SG)
    op_r = np.ascontiguousarray(out_proj_w.T.astype(ml_dtypes.bfloat16))  # [DI, D]
    convw_r = np.ascontiguousarray(
        conv_w.reshape(KC, 128, 4).transpose(1, 0, 2).reshape(128, KC * 4))
    convb_r = np.ascontiguousarray(conv_b.reshape(KC, 128).T)
    gateb_r = np.ascontiguousarray(gate_b.reshape(KC, 128).T)
    gatebn_r = np.ascontiguousarray(-gateb_r)
    inbx_r = np.ascontiguousarray(inb[:DI].reshape(KC, 128).T)
    inbz_r = np.ascontiguousarray(inb[DI:].reshape(KC, 128).T)

    in_maps = []
    for core in range(8):
        b, j = core // 4, core % 4
        xs = np.zeros((NT, D), np.float32)
        start = j * CHUNK - W
        mask = np.ones((1, NT), ml_dtypes.bfloat16)
        if j == 0:
            xs[W:] = x[b, 0:CHUNK]
            mask[0, :W] = 0.0
        else:
            xs[:] = x[b, start:start + NT]
        in_maps.append({
            "x": np.ascontiguousarray(xs), "w1x": w1x_r, "w1z": w1z_r,
            "gw": gw_r, "opw": op_r, "convw": convw_r, "convb": convb_r,
            "gateb": gateb_r, "gatebn": gatebn_r,
            "inbx": inbx_r, "inbz": inbz_r, "mask": mask,
        })
    return in_maps


def kernel(x, norm_w, norm_b, in_proj_w, conv_w, conv_b, gate_w, gate_b,
           out_proj_w, _trace=False, _collect=None):
    x = np.asarray(x, np.float32)
    if "nc" not in _cache:
        _cache["nc"] = _build()
    nc = _cache["nc"]
    in_maps = _prep_host(
        x, np.asarray(norm_w, np.float32), np.asarray(norm_b, np.float32),
        np.asarray(in_proj_w, np.float32), np.asarray(conv_w, np.float32),
        np.asarray(conv_b, np.float32), np.asarray(gate_w, np.float32),
        np.asarray(gate_b, np.float32), np.asarray(out_proj_w, np.float32))
    res = run_bass_kernel_spmd(nc, in_maps, core_ids=list(range(8)), trace=_trace)
    if _collect is not None:
        _collect.append(res)
    out = np.empty((B, L, D), np.float32)
    for core in range(8):
        b, j = core // 4, core % 4
        out[b, j * CHUNK:(j + 1) * CHUNK] = res.results[core]["out"]
    return out
